# revision 15
# baseline (speedup 1.0000x reference)
"""Mixtral MoE layer (top-2 of 8 experts) on 8 Trainium2 NeuronCores.

Strategy: expert parallelism with a sharded router.
Core e owns expert e's weights AND token shard e (tokens [e*2048,(e+1)*2048)).
  1. Router (sharded, exact fp32): each core computes logitsT [8, 2048] for
     its own shard only, AllGather -> lg_all [64, 2048] on every core.
  2. Each core computes, for its own expert, the top-2 mask + combine weight
     (sigmoid trick) for ALL 16384 tokens, plus a per-(expert,shard)-bucket
     compaction rank (matmul prefix sums; bucket s base = s*BCAP).
     Only 8-byte metadata rows [token_id, combine] are scattered to meta[];
     the scatters are interleaved with FFN chunks and gated by per-shard
     fences so FFN chunk c never waits on unrelated shards.
  3. FFN over compact slots in chunks with one-chunk lookahead (next chunk's
     meta load, h-row gathers and PE transposes are emitted mid-chunk):
     stage A (bf16, w1/w3 streamed from a pre-converted bf16 DRAM copy),
     stage B (bf16, w2 resident in SBUF), output scaled by combine and
     scattered to scat[token_id] (pads go to a trash row; gathers clamp
     pad ids to T-1 so no uninitialized SBUF is ever read).
  4. Output: 8 slab-ReduceScatters (one per shard), each issued as soon as
     the chunks covering that shard's bucket are done -> overlapped with FFN
     compute; per-slab output copies also overlap. Host reassembles the
     resulting row distribution.
"""
import sys

sys.path.insert(0, "/opt/trn_rl_repo")

import numpy as np

import concourse.bass as bass
import concourse.mybir as mybir
from concourse import bacc
from concourse.tile import TileContext
from concourse.tile_rust import add_dep_helper
from concourse.masks import make_identity
from concourse.bass_utils import run_bass_kernel_spmd

F32 = mybir.dt.float32
F32R = mybir.dt.float32r
BF16 = mybir.dt.bfloat16
I32 = mybir.dt.int32
AF = mybir.ActivationFunctionType
P = 128


def build_kernel(T=16384, H=1024, FF=3584, E=8, BCAP=576, CH=512, n_cores=8):
    NT = T // P          # 128 global token tiles
    KH = H // P          # 8 contraction tiles over H
    KF = FF // P         # 28 f tiles
    S = n_cores          # shards
    TS = T // S          # 2048 tokens per shard
    NTS = TS // P        # 16 token tiles per shard
    TCAP = BCAP * S      # 4608 compact slots
    NCT = TCAP // P      # 36 compact token tiles
    NCH = TCAP // CH     # 9 chunks
    CT = CH // P         # 4 token tiles per chunk
    PS = TS // S         # 256 rows per slab-RS piece
    BIG = 1.0e9
    TRASH = float(T)     # scat row for capacity-pad slots

    nc = bacc.Bacc(num_devices=n_cores, num_swdge_queues=4)

    h_ext = nc.dram_tensor("h", [T, H], F32, kind="ExternalInput")
    hs_ext = nc.dram_tensor("hs", [TS, H], F32, kind="ExternalInput")
    gw_ext = nc.dram_tensor("gate_w", [H, E], F32, kind="ExternalInput")
    w1_ext = nc.dram_tensor("w1l", [H, FF], F32, kind="ExternalInput")
    w3_ext = nc.dram_tensor("w3l", [H, FF], F32, kind="ExternalInput")
    w2_ext = nc.dram_tensor("w2l", [FF, H], F32, kind="ExternalInput")
    oh64_ext = nc.dram_tensor("oh64", [P, S * E], F32, kind="ExternalInput")
    out_ext = nc.dram_tensor("out_shard", [TS, H], F32, kind="ExternalOutput")

    meta = nc.dram_tensor("meta", [TCAP, 2], F32)
    lg_in = nc.dram_tensor("lg_in", [E, TS], F32)
    lg_all = nc.dram_tensor("lg_all", [S * E, TS], F32, addr_space="Shared")
    scat = nc.dram_tensor("scat", [T + P, H], BF16)
    w1bd = nc.dram_tensor("w1bd", [KF, P, KH * P], BF16)
    w3bd = nc.dram_tensor("w3bd", [KF, P, KH * P], BF16)
    rs_out = nc.dram_tensor("rs_out", [TS, H], BF16)

    tok_ids = np.arange(T, dtype=np.float32).reshape(NT, P).T.copy()  # [P, NT]
    tok_const = nc.inline_tensor(tok_ids, name="tok_ids")
    ustrict_np = np.triu(np.ones((P, P), dtype=np.float32), 1)  # [k,m]=1 iff k<m
    ustrict_const = nc.inline_tensor(ustrict_np, name="ustrict")
    # segmented-scan helpers: zmask=0 at each shard's first tile, bucket bases
    zmask_np = np.ones((1, NT), dtype=np.float32)
    zmask_np[0, ::NTS] = 0.0
    zmask_const = nc.inline_tensor(zmask_np, name="zmask")
    bbase_np = (np.arange(NT, dtype=np.float32) // NTS * BCAP).reshape(1, NT)
    bbase_const = nc.inline_tensor(bbase_np, name="bbase")

    # bucket s (compact rows [s*BCAP,(s+1)*BCAP)) is complete after this chunk
    slab_ready_chunk = [((s + 1) * BCAP - 1) // CH for s in range(S)]

    with TileContext(nc) as tc:
        with tc.tile_pool(name="const", bufs=1) as cpool, \
             tc.tile_pool(name="stage", bufs=2) as spool, \
             tc.tile_pool(name="keep", bufs=1) as kpool:
            ident = cpool.tile([P, P], F32)
            make_identity(nc, ident[:])
            ustrict = cpool.tile([P, P], F32)
            nc.sync.dma_start(out=ustrict[:], in_=ustrict_const[:])
            tok_slab = cpool.tile([P, NT], F32)
            nc.sync.dma_start(out=tok_slab[:], in_=tok_const[:])
            ones_col = cpool.tile([P, 1], F32)
            nc.vector.memset(ones_col[:], 1.0)
            ones_row = cpool.tile([1, P], F32)
            nc.vector.memset(ones_row[:], 1.0)
            gw_sb = cpool.tile([P, KH, E], F32)
            nc.sync.dma_start(out=gw_sb[:],
                              in_=gw_ext[:].rearrange("(k p) e -> p k e", p=P))
            oh3 = cpool.tile([P, S, E], F32)
            nc.sync.dma_start(out=oh3[:],
                              in_=oh64_ext[:].rearrange("p (s e) -> p s e", s=S))
            zrow2 = cpool.tile([P, 2], F32)
            nc.vector.memset(zrow2[:], 0.0)
            nc.vector.memset(zrow2[:, 0:1], TRASH)
            zrow_b = cpool.tile([P, H], BF16)
            nc.vector.memset(zrow_b[:], 0.0)
            zer_row = cpool.tile([1, P], F32)
            nc.vector.memset(zer_row[:], 0.0)
            zmask = cpool.tile([1, NT], F32)
            nc.sync.dma_start(out=zmask[:], in_=zmask_const[:])
            bbase = cpool.tile([1, NT], F32)
            nc.sync.dma_start(out=bbase[:], in_=bbase_const[:])
            w2b = cpool.tile([P, KF, H], BF16)  # resident stage-B weights

            # meta zero-fill must precede the rank scatters
            nc.gpsimd.dma_start(
                out=meta[:].rearrange("(a p) w -> p a w", p=P),
                in_=zrow2[:, None, :].to_broadcast([P, NCT, 2]))

            # -------- router: own shard logits -> AllGather --------
            with tc.tile_pool(name="rth", bufs=12) as htpool, \
                 tc.tile_pool(name="rlg", bufs=2) as lgpool, \
                 tc.tile_pool(name="rt4", bufs=2) as h4pool, \
                 tc.tile_pool(name="rps", bufs=2, space="PSUM") as rpsum, \
                 tc.tile_pool(name="lgps", bufs=2, space="PSUM") as lgpsum:
                SG = 4  # token tiles per 512-token strip
                hts = []
                for i in range(NTS):
                    ht = htpool.tile([P, H], F32, tag="ht")
                    eng = nc.sync if i % 2 == 0 else nc.scalar
                    eng.dma_start(out=ht[:], in_=hs_ext[i * P:(i + 1) * P, :])
                    hts.append(ht)
                for st in range(NTS // SG):
                    hT4 = h4pool.tile([P, KH, SG * P], F32, tag="hT4")
                    for j in range(SG):
                        i = st * SG + j
                        trp = rpsum.tile([P, KH, P], F32, tag="trp")
                        for k in range(KH):
                            nc.tensor.transpose(out=trp[:, k],
                                                in_=hts[i][:, k * P:(k + 1) * P],
                                                identity=ident[:])
                        nc.vector.tensor_copy(out=hT4[:, :, j * P:(j + 1) * P],
                                              in_=trp[:])
                    lgT = lgpsum.tile([E, SG * P], F32, tag="lgT")
                    for k in range(KH):
                        nc.tensor.matmul(lgT[:], lhsT=gw_sb[:, k], rhs=hT4[:, k],
                                         start=(k == 0), stop=(k == KH - 1))
                    lgT_sb = lgpool.tile([E, SG * P], F32, tag="lgT_sb")
                    nc.vector.tensor_copy(out=lgT_sb[:], in_=lgT[:])
                    nc.sync.dma_start(
                        out=lg_in[:, st * SG * P:(st + 1) * SG * P],
                        in_=lgT_sb[:])

            ag = nc.gpsimd.collective_compute(
                "AllGather", mybir.AluOpType.bypass,
                replica_groups=[list(range(n_cores))],
                ins=[lg_in[:]], outs=[lg_all[:]])

            # ---- weight pre-conversion fp32 -> bf16 (w1/w3 first, 2 queues) --
            for f in range(KF):
                for (wext, wdst, eng, tg) in ((w1_ext, w1bd, nc.sync, "a"),
                                              (w3_ext, w3bd, nc.scalar, "b")):
                    wst = spool.tile([P, KH, P], F32, tag="wst" + tg)
                    eng.dma_start(
                        out=wst[:],
                        in_=wext[:, f * P:(f + 1) * P].rearrange(
                            "(k p) m -> p k m", p=P))
                    wbt = spool.tile([P, KH * P], BF16, tag="wbt" + tg)
                    nc.vector.tensor_copy(
                        out=wbt[:].rearrange("p (k m) -> p k m", k=KH),
                        in_=wst[:])
                    eng.dma_start(out=wdst[f], in_=wbt[:])
            for f in range(KF):
                w2st = spool.tile([P, H], F32, tag="w2st")
                nc.sync.dma_start(out=w2st[:], in_=w2_ext[f * P:(f + 1) * P, :])
                nc.vector.tensor_copy(out=w2b[:, f], in_=w2st[:])

            # -------- zero-fill scat (runs on SWDGE during router/combine) --
            ZB = 4
            NSC = (T + P) // P
            for r in range(NSC // ZB):
                nc.gpsimd.dma_start(
                    out=scat[r * P * ZB:(r + 1) * P * ZB, :].rearrange(
                        "(a p) w -> p a w", p=P),
                    in_=zrow_b[:, None, :].to_broadcast([P, ZB, H]))
            for r in range((NSC // ZB) * ZB, NSC):
                nc.gpsimd.dma_start(out=scat[r * P:(r + 1) * P, :], in_=zrow_b[:])

            # -------- combine + mask + bucket ranks for ALL tokens --------
            # long-lived outputs (kpool): rank_i, pays; the rest is scratch
            rank_i = kpool.tile([P, NT], I32)
            pays = kpool.tile([P, NT, 2], F32)
            with tc.tile_pool(name="cmb", bufs=1) as mpool, \
                 tc.tile_pool(name="cps", bufs=2, space="PSUM") as cpsum:
                lg_sb = mpool.tile([S * E, TS], F32)
                ldg = nc.scalar.dma_start(out=lg_sb[:], in_=lg_all[:])
                add_dep_helper(ldg.ins, ag.ins, True, "allgather done")
                # lgJ[p, j, s, e]: logit of expert e for token s*2048+j*128+p
                lgJ = mpool.tile([P, NTS, S, E], F32)
                for j in range(NTS):
                    lt = cpsum.tile([P, S * E], F32, tag="lt")
                    nc.tensor.transpose(out=lt[:],
                                        in_=lg_sb[:, j * P:(j + 1) * P],
                                        identity=ident[0:S * E, 0:S * E])
                    nc.scalar.copy(
                        out=lgJ[:, j],
                        in_=lt[:].rearrange("p (s e) -> p s e", s=S))

                # per-token top-8 of own-shard logits; nt = s*NTS + j
                mx = mpool.tile([P, NT, 8], F32)
                for j in range(NTS):
                    for s in range(S):
                        nc.vector.max(out=mx[:, s * NTS + j],
                                      in_=lgJ[:, j, s])

                # own-expert logit: le[:, s*NTS+j] = sum_e lgJ[:,j,s,e]*oh3[:,s,e]
                tmp4 = mpool.tile([P, NTS, S, E], F32)
                nc.vector.tensor_mul(
                    out=tmp4[:], in0=lgJ[:],
                    in1=oh3[:, None, :, :].to_broadcast([P, NTS, S, E]))
                leJ = mpool.tile([P, NTS, S], F32)
                nc.vector.tensor_reduce(out=leJ[:], in_=tmp4[:],
                                        axis=mybir.AxisListType.X,
                                        op=mybir.AluOpType.add)
                le = mpool.tile([P, NT], F32)
                for s in range(S):
                    nc.vector.tensor_copy(out=le[:, s * NTS:(s + 1) * NTS],
                                          in_=leJ[:, :, s])

                m1 = mx[:, :, 0]
                m2 = mx[:, :, 1]
                msum = mpool.tile([P, NT], F32)
                nc.vector.tensor_add(out=msum[:], in0=m1, in1=m2)
                sgin = mpool.tile([P, NT], F32)
                nc.vector.tensor_scalar_mul(sgin[:], le[:], 2.0)
                nc.vector.tensor_sub(out=sgin[:], in0=sgin[:], in1=msum[:])
                sig = mpool.tile([P, NT], F32)
                nc.scalar.activation(sig[:], sgin[:], AF.Sigmoid)
                eq1 = mpool.tile([P, NT], F32)
                eq2 = mpool.tile([P, NT], F32)
                nc.vector.tensor_tensor(out=eq1[:], in0=le[:], in1=m1,
                                        op=mybir.AluOpType.is_equal)
                nc.vector.tensor_tensor(out=eq2[:], in0=le[:], in1=m2,
                                        op=mybir.AluOpType.is_equal)
                mask = mpool.tile([P, NT], F32)
                nc.vector.tensor_add(out=mask[:], in0=eq1[:], in1=eq2[:])
                comb = mpool.tile([P, NT], F32)
                nc.vector.tensor_mul(out=comb[:], in0=mask[:], in1=sig[:])

                nc.vector.tensor_copy(out=pays[:, :, 0], in_=tok_slab[:])
                nc.vector.tensor_copy(out=pays[:, :, 1], in_=comb[:])

                # bucket-local ranks, all shards at once via segmented scan:
                # incl_j = zmask_j * incl_{j-1} + cs_j  (resets at shard starts)
                cs_row = mpool.tile([1, NT], F32)
                incl_row = mpool.tile([1, NT], F32)
                csum_ps = cpsum.tile([1, NT], F32, tag="c1")
                nc.tensor.matmul(csum_ps[:], lhsT=ones_col[:], rhs=mask[:],
                                 start=True, stop=True)
                nc.vector.tensor_copy(out=cs_row[:], in_=csum_ps[:])
                nc.vector.tensor_tensor_scan(out=incl_row[:],
                                             data0=zmask[:], data1=cs_row[:],
                                             initial=0.0,
                                             op0=mybir.AluOpType.mult,
                                             op1=mybir.AluOpType.add)
                cpref = mpool.tile([1, NT], F32)
                nc.vector.tensor_sub(out=cpref[:], in0=incl_row[:], in1=cs_row[:])
                nc.vector.tensor_add(out=cpref[:], in0=cpref[:], in1=bbase[:])
                rank_ps = cpsum.tile([P, NT], F32, tag="rk")
                nc.tensor.matmul(rank_ps[:], lhsT=ustrict[:], rhs=mask[:],
                                 start=True, stop=False)
                nc.tensor.matmul(rank_ps[:], lhsT=ones_row[:], rhs=cpref[:],
                                 start=False, stop=True)
                pad_off = mpool.tile([P, NT], F32)
                nc.vector.tensor_scalar(out=pad_off[:], in0=mask[:],
                                        scalar1=-BIG, scalar2=BIG,
                                        op0=mybir.AluOpType.mult,
                                        op1=mybir.AluOpType.add)
                rank_f = mpool.tile([P, NT], F32)
                nc.vector.tensor_add(out=rank_f[:], in0=rank_ps[:],
                                     in1=pad_off[:])
                nc.vector.tensor_copy(out=rank_i[:], in_=rank_f[:])

            # -------- FFN over compact slots, shard scatters interleaved ----
            fences = {}

            def emit_shard_scatters(s):
                shard_scatters = []
                for j in range(NTS):
                    nt = s * NTS + j
                    claim = bass.AP(
                        tensor=meta[0:P, :].tensor, offset=0,
                        ap=meta[0:P, :].ap,
                        dep_tracking_offset=(nt % NCT) * P * 2)
                    sc = nc.gpsimd.indirect_dma_start(
                        out=claim,
                        out_offset=bass.IndirectOffsetOnAxis(
                            ap=rank_i[:, nt:nt + 1], axis=0),
                        in_=pays[:, nt], in_offset=None,
                        bounds_check=TCAP - 1, oob_is_err=False)
                    sc.ins.queue = "qPoolDynamic" + str(nt % 4 or '')
                    shard_scatters.append(sc.ins)
                fence = nc.gpsimd.nop(hint=f"meta_fence_{s}", nofuse=True)
                for si in shard_scatters:
                    add_dep_helper(fence.ins, si, True, "meta scatter fence")
                fences[s] = fence

            rs_insts = {}
            out_scatter_insts = [[] for _ in range(S)]
            with tc.tile_pool(name="hcp", bufs=CT + 2) as hcpool, \
                 tc.tile_pool(name="mtp", bufs=2) as mtpool, \
                 tc.tile_pool(name="fp", bufs=2) as fpool, \
                 tc.tile_pool(name="wp", bufs=4) as wpool, \
                 tc.tile_pool(name="gp", bufs=KF) as gpool, \
                 tc.tile_pool(name="op", bufs=3) as opool, \
                 tc.tile_pool(name="ftps", bufs=1, space="PSUM") as ftrpsum, \
                 tc.tile_pool(name="fps", bufs=2, space="PSUM") as fpsum, \
                 tc.tile_pool(name="ops", bufs=1, space="PSUM") as opsum:

                def emit_chunk_load(c):
                    """meta load + h gathers + PE transposes for chunk c."""
                    r0 = c * CH
                    lo_sh = r0 // BCAP
                    hi_sh = (r0 + CH - 1) // BCAP
                    mt = mtpool.tile([P, CT, 2], F32, tag="mt")
                    ld = nc.scalar.dma_start(
                        out=mt[:],
                        in_=meta[r0:r0 + CH, :].rearrange(
                            "(a p) w -> p a w", p=P))
                    for s in range(lo_sh, hi_sh + 1):
                        add_dep_helper(ld.ins, fences[s].ins, True, "meta fence")
                    idx = mtpool.tile([P, CT], I32, tag="idx")
                    nc.vector.tensor_copy(out=idx[:], in_=mt[:, :, 0])
                    idxg_f = mtpool.tile([P, CT], F32, tag="idxgf")
                    nc.vector.tensor_scalar_min(idxg_f[:], mt[:, :, 0],
                                                float(T - 1))
                    idxg = mtpool.tile([P, CT], I32, tag="idxg")
                    nc.vector.tensor_copy(out=idxg[:], in_=idxg_f[:])
                    hcts = []
                    for t in range(CT):
                        hct = hcpool.tile([P, H], F32, tag="hc")
                        g = nc.gpsimd.indirect_dma_start(
                            out=hct[:], out_offset=None,
                            in_=bass.AP(tensor=h_ext[0:P, :].tensor, offset=0,
                                        ap=h_ext[0:P, :].ap,
                                        dep_tracking_offset=0),
                            in_offset=bass.IndirectOffsetOnAxis(
                                ap=idxg[:, t:t + 1], axis=0),
                            bounds_check=T - 1, oob_is_err=False)
                        g.ins.queue = "qPoolDynamic" + str((c * CT + t) % 4 or '')
                        hcts.append(hct)
                    hTr = fpool.tile([P, KH, CH], BF16, tag="hTr")
                    for t in range(CT):
                        trp = ftrpsum.tile([P, KH, P], F32, tag="ftr")
                        for k in range(KH):
                            nc.tensor.transpose(
                                out=trp[:, k],
                                in_=hcts[t][:, k * P:(k + 1) * P],
                                identity=ident[:])
                        nc.vector.tensor_copy(out=hTr[:, :, t * P:(t + 1) * P],
                                              in_=trp[:])
                    return mt, idx, hTr

                # prologue: shard 0 scatters, chunk 0 load, shard 1 scatters
                emit_shard_scatters(0)
                cur = emit_chunk_load(0)
                emit_shard_scatters(1)

                for c in range(NCH):
                    mt, idx, hTr = cur

                    # stage A: G^T tiles [f, tokens], bf16 streamed weights
                    gts = []
                    for f in range(KF):
                        w1s = wpool.tile([P, KH, P], BF16, tag="w1s")
                        nc.sync.dma_start(
                            out=w1s[:],
                            in_=w1bd[f].rearrange("p (k m) -> p k m", k=KH))
                        w3s = wpool.tile([P, KH, P], BF16, tag="w3s")
                        nc.sync.dma_start(
                            out=w3s[:],
                            in_=w3bd[f].rearrange("p (k m) -> p k m", k=KH))
                        x1 = fpsum.tile([P, CH], F32, tag="x1")
                        x3 = fpsum.tile([P, CH], F32, tag="x3")
                        for k in range(KH):
                            nc.tensor.matmul(x1[:], lhsT=w1s[:, k], rhs=hTr[:, k],
                                             start=(k == 0), stop=(k == KH - 1))
                        for k in range(KH):
                            nc.tensor.matmul(x3[:], lhsT=w3s[:, k], rhs=hTr[:, k],
                                             start=(k == 0), stop=(k == KH - 1))
                        gate = fpool.tile([P, CH], F32, tag="gate")
                        nc.scalar.activation(gate[:], x1[:], AF.Silu)
                        gt = gpool.tile([P, CH], BF16, tag="G")
                        nc.vector.tensor_mul(out=gt[:], in0=gate[:], in1=x3[:])
                        gts.append(gt)

                    # lookahead: next chunk's meta/gathers/transposes now, so
                    # its hTr copies complete during stage B; deferred shard
                    # scatters go after the gathers on the gpsimd engine
                    if c + 1 < NCH:
                        cur = emit_chunk_load(c + 1)
                    if c + 2 < S:
                        emit_shard_scatters(c + 2)

                    # stage B: out rows, scaled by combine, scattered to scat
                    for t in range(CT):
                        o = opsum.tile([P, H], F32, tag="o")
                        for f in range(KF):
                            for hh in range(2):
                                nc.tensor.matmul(
                                    o[:, hh * 512:(hh + 1) * 512],
                                    lhsT=gts[f][:, t * P:(t + 1) * P],
                                    rhs=w2b[:, f, hh * 512:(hh + 1) * 512],
                                    start=(f == 0), stop=(f == KF - 1))
                        osb = opool.tile([P, H], BF16, tag="osb")
                        nc.vector.tensor_scalar_mul(osb[:], o[:], mt[:, t, 1:2])
                        gi = c * CT + t
                        lo_s = (gi * P) // BCAP
                        hi_s = min((gi * P + P - 1) // BCAP, S - 1)
                        if lo_s == hi_s:
                            doff = (lo_s * TS + (gi * P) % BCAP) * H
                        else:
                            doff = (hi_s * TS - P // 2) * H
                        oclaim = bass.AP(
                            tensor=scat[0:P, :].tensor, offset=0,
                            ap=scat[0:P, :].ap,
                            dep_tracking_offset=doff)
                        sco = nc.gpsimd.indirect_dma_start(
                            out=oclaim,
                            out_offset=bass.IndirectOffsetOnAxis(
                                ap=idx[:, t:t + 1], axis=0),
                            in_=osb[:], in_offset=None,
                            bounds_check=T + P - 1, oob_is_err=False)
                        sco.ins.queue = "qPoolDynamic" + str(gi % 4 or '')
                        for s in range(lo_s, hi_s + 1):
                            out_scatter_insts[s].append(sco.ins)

                    # issue slab ReduceScatters that became ready + out copies
                    for s in range(S):
                        if slab_ready_chunk[s] == c:
                            rs = nc.gpsimd.collective_compute(
                                "ReduceScatter", mybir.AluOpType.add,
                                replica_groups=[list(range(n_cores))],
                                ins=[scat[s * TS:(s + 1) * TS, :]],
                                outs=[rs_out[s * PS:(s + 1) * PS, :]])
                            for si in out_scatter_insts[s]:
                                add_dep_helper(rs.ins, si, True, "slab scatter")
                            rs_insts[s] = rs
                            for r in range(PS // P):
                                b0 = s * PS + r * P
                                oct_ = spool.tile([P, H], BF16, tag="oct")
                                ldo = nc.scalar.dma_start(
                                    out=oct_[:], in_=rs_out[b0:b0 + P, :])
                                add_dep_helper(ldo.ins, rs.ins, True, "rs done")
                                octf = spool.tile([P, H], F32, tag="octf")
                                nc.vector.tensor_copy(out=octf[:], in_=oct_[:])
                                nc.scalar.dma_start(out=out_ext[b0:b0 + P, :],
                                                    in_=octf[:])

    nc.finalize()
    return nc


def make_in_maps(hidden_states, gate_w, w1, w3, w2, n_cores=8):
    T, H = hidden_states.shape
    E = w1.shape[0]
    TS = T // n_cores
    h_full = np.ascontiguousarray(hidden_states, dtype=np.float32)
    in_maps = []
    for e in range(n_cores):
        oh64 = np.zeros(n_cores * E, dtype=np.float32)
        oh64[np.arange(n_cores) * E + e] = 1.0  # expert e, in every shard block
        in_maps.append({
            "h": h_full,
            "hs": np.ascontiguousarray(h_full[e * TS:(e + 1) * TS]),
            "gate_w": np.ascontiguousarray(gate_w, dtype=np.float32),
            "w1l": np.ascontiguousarray(w1[e], dtype=np.float32),
            "w3l": np.ascontiguousarray(w3[e], dtype=np.float32),
            "w2l": np.ascontiguousarray(w2[e], dtype=np.float32),
            "oh64": np.tile(oh64, (128, 1)),
        })
    return in_maps


def assemble_output(results, T=16384, H=1024, n_cores=8):
    # slab-RS s gives core r rows [s*2048 + r*256, s*2048 + (r+1)*256)
    S = n_cores
    TS = T // S
    PS = TS // S
    full = np.empty((T, H), dtype=np.float32)
    for r in range(n_cores):
        shard = np.asarray(results[r]["out_shard"])
        for s in range(S):
            full[s * TS + r * PS:s * TS + (r + 1) * PS] = \
                shard[s * PS:(s + 1) * PS]
    return full


def kernel(hidden_states, gate_w, w1, w3, w2):
    T, H = hidden_states.shape
    E, _, FF = w1.shape
    n_cores = 8
    nc = build_kernel(T=T, H=H, FF=FF, E=E, n_cores=n_cores)
    in_maps = make_in_maps(hidden_states, gate_w, w1, w3, w2, n_cores)
    res = run_bass_kernel_spmd(nc, in_maps, list(range(n_cores))).results
    return assemble_output(res, T=T, H=H, n_cores=n_cores)


if __name__ == "__main__":
    nc = build_kernel()
    print("built", len(nc.inst_map), "instructions")


# revision 16
# speedup vs baseline: 1.0072x; 1.0072x over previous
"""Mixtral MoE layer (top-2 of 8 experts) on 8 Trainium2 NeuronCores.

Strategy: expert parallelism with a sharded router.
Core e owns expert e's weights AND token shard e (tokens [e*2048,(e+1)*2048)).
  1. Router (sharded, exact fp32): each core computes logitsT [8, 2048] for
     its own shard only, AllGather -> lg_all [64, 2048] on every core.
  2. Each core computes, for its own expert, the top-2 mask + combine weight
     (sigmoid trick) for ALL 16384 tokens, plus a per-(expert,shard)-bucket
     compaction rank (matmul prefix sums; bucket s base = s*BCAP).
     Only 8-byte metadata rows [token_id, combine] are scattered to meta[];
     the scatters are interleaved with FFN chunks and gated by per-shard
     fences so FFN chunk c never waits on unrelated shards.
  3. FFN over compact slots in chunks with one-chunk lookahead (next chunk's
     meta load, h-row gathers and PE transposes are emitted mid-chunk):
     stage A (bf16, w1/w3 streamed from a pre-converted bf16 DRAM copy),
     stage B (bf16, w2 resident in SBUF), output scaled by combine and
     scattered to scat[token_id] (pads go to a trash row; gathers clamp
     pad ids to T-1 so no uninitialized SBUF is ever read).
  4. Output: 8 slab-ReduceScatters (one per shard), each issued as soon as
     the chunks covering that shard's bucket are done -> overlapped with FFN
     compute; per-slab output copies also overlap. Host reassembles the
     resulting row distribution.
"""
import sys

sys.path.insert(0, "/opt/trn_rl_repo")

import numpy as np

import concourse.bass as bass
import concourse.mybir as mybir
from concourse import bacc
from concourse.tile import TileContext
from concourse.tile_rust import add_dep_helper
from concourse.masks import make_identity
from concourse.bass_utils import run_bass_kernel_spmd

F32 = mybir.dt.float32
F32R = mybir.dt.float32r
BF16 = mybir.dt.bfloat16
I32 = mybir.dt.int32
AF = mybir.ActivationFunctionType
P = 128


def build_kernel(T=16384, H=1024, FF=3584, E=8, BCAP=576, CH=512, n_cores=8):
    NT = T // P          # 128 global token tiles
    KH = H // P          # 8 contraction tiles over H
    KF = FF // P         # 28 f tiles
    S = n_cores          # shards
    TS = T // S          # 2048 tokens per shard
    NTS = TS // P        # 16 token tiles per shard
    TCAP = BCAP * S      # 4608 compact slots
    NCT = TCAP // P      # 36 compact token tiles
    NCH = TCAP // CH     # 9 chunks
    CT = CH // P         # 4 token tiles per chunk
    PS = TS // S         # 256 rows per slab-RS piece
    BIG = 1.0e9
    TRASH = float(T)     # scat row for capacity-pad slots

    nc = bacc.Bacc(num_devices=n_cores, num_swdge_queues=4)

    h_ext = nc.dram_tensor("h", [T, H], F32, kind="ExternalInput")
    hs_ext = nc.dram_tensor("hs", [TS, H], F32, kind="ExternalInput")
    gw_ext = nc.dram_tensor("gate_w", [H, E], F32, kind="ExternalInput")
    w1_ext = nc.dram_tensor("w1l", [H, FF], F32, kind="ExternalInput")
    w3_ext = nc.dram_tensor("w3l", [H, FF], F32, kind="ExternalInput")
    w2_ext = nc.dram_tensor("w2l", [FF, H], F32, kind="ExternalInput")
    oh64_ext = nc.dram_tensor("oh64", [P, S * E], F32, kind="ExternalInput")
    out_ext = nc.dram_tensor("out_shard", [TS, H], F32, kind="ExternalOutput")

    meta = nc.dram_tensor("meta", [TCAP, 2], F32)
    lg_in = nc.dram_tensor("lg_in", [E, TS], F32)
    lg_all = nc.dram_tensor("lg_all", [S * E, TS], F32, addr_space="Shared")
    scat = nc.dram_tensor("scat", [T + P, H], BF16)
    w1bd = nc.dram_tensor("w1bd", [KF, P, KH * P], BF16)
    w3bd = nc.dram_tensor("w3bd", [KF, P, KH * P], BF16)
    rs_out = nc.dram_tensor("rs_out", [TS, H], BF16)

    tok_ids = np.arange(T, dtype=np.float32).reshape(NT, P).T.copy()  # [P, NT]
    tok_const = nc.inline_tensor(tok_ids, name="tok_ids")
    ustrict_np = np.triu(np.ones((P, P), dtype=np.float32), 1)  # [k,m]=1 iff k<m
    ustrict_const = nc.inline_tensor(ustrict_np, name="ustrict")
    # segmented-scan helpers: zmask=0 at each shard's first tile, bucket bases
    zmask_np = np.ones((1, NT), dtype=np.float32)
    zmask_np[0, ::NTS] = 0.0
    zmask_const = nc.inline_tensor(zmask_np, name="zmask")
    bbase_np = (np.arange(NT, dtype=np.float32) // NTS * BCAP).reshape(1, NT)
    bbase_const = nc.inline_tensor(bbase_np, name="bbase")

    # bucket s (compact rows [s*BCAP,(s+1)*BCAP)) is complete after this chunk
    slab_ready_chunk = [((s + 1) * BCAP - 1) // CH for s in range(S)]

    with TileContext(nc) as tc:
        with tc.tile_pool(name="const", bufs=1) as cpool, \
             tc.tile_pool(name="stage", bufs=2) as spool, \
             tc.tile_pool(name="keep", bufs=1) as kpool:
            ident = cpool.tile([P, P], F32)
            make_identity(nc, ident[:])
            ustrict = cpool.tile([P, P], F32)
            nc.sync.dma_start(out=ustrict[:], in_=ustrict_const[:])
            tok_slab = cpool.tile([P, NT], F32)
            nc.sync.dma_start(out=tok_slab[:], in_=tok_const[:])
            ones_col = cpool.tile([P, 1], F32)
            nc.vector.memset(ones_col[:], 1.0)
            ones_row = cpool.tile([1, P], F32)
            nc.vector.memset(ones_row[:], 1.0)
            gw_sb = cpool.tile([P, KH, E], F32)
            nc.sync.dma_start(out=gw_sb[:],
                              in_=gw_ext[:].rearrange("(k p) e -> p k e", p=P))
            oh3 = cpool.tile([P, S, E], F32)
            nc.sync.dma_start(out=oh3[:],
                              in_=oh64_ext[:].rearrange("p (s e) -> p s e", s=S))
            zrow2 = cpool.tile([P, 2], F32)
            nc.vector.memset(zrow2[:], 0.0)
            nc.vector.memset(zrow2[:, 0:1], TRASH)
            zrow_b = cpool.tile([P, H], BF16)
            nc.vector.memset(zrow_b[:], 0.0)
            zer_row = cpool.tile([1, P], F32)
            nc.vector.memset(zer_row[:], 0.0)
            zmask = cpool.tile([1, NT], F32)
            nc.sync.dma_start(out=zmask[:], in_=zmask_const[:])
            bbase = cpool.tile([1, NT], F32)
            nc.sync.dma_start(out=bbase[:], in_=bbase_const[:])
            w2b = cpool.tile([P, KF, H], BF16)  # resident stage-B weights

            # meta zero-fill must precede the rank scatters
            nc.gpsimd.dma_start(
                out=meta[:].rearrange("(a p) w -> p a w", p=P),
                in_=zrow2[:, None, :].to_broadcast([P, NCT, 2]))

            # -------- router: own shard logits -> AllGather --------
            with tc.tile_pool(name="rth", bufs=12) as htpool, \
                 tc.tile_pool(name="rlg", bufs=2) as lgpool, \
                 tc.tile_pool(name="rt4", bufs=2) as h4pool, \
                 tc.tile_pool(name="rps", bufs=2, space="PSUM") as rpsum, \
                 tc.tile_pool(name="lgps", bufs=2, space="PSUM") as lgpsum:
                SG = 4  # token tiles per 512-token strip
                hts = []
                for i in range(NTS):
                    ht = htpool.tile([P, H], F32, tag="ht")
                    eng = nc.sync if i % 2 == 0 else nc.scalar
                    eng.dma_start(out=ht[:], in_=hs_ext[i * P:(i + 1) * P, :])
                    hts.append(ht)
                for st in range(NTS // SG):
                    hT4 = h4pool.tile([P, KH, SG * P], F32, tag="hT4")
                    for j in range(SG):
                        i = st * SG + j
                        trp = rpsum.tile([P, KH, P], F32, tag="trp")
                        for k in range(KH):
                            nc.tensor.transpose(out=trp[:, k],
                                                in_=hts[i][:, k * P:(k + 1) * P],
                                                identity=ident[:])
                        nc.vector.tensor_copy(out=hT4[:, :, j * P:(j + 1) * P],
                                              in_=trp[:])
                    lgT = lgpsum.tile([E, SG * P], F32, tag="lgT")
                    for k in range(KH):
                        nc.tensor.matmul(lgT[:], lhsT=gw_sb[:, k], rhs=hT4[:, k],
                                         start=(k == 0), stop=(k == KH - 1))
                    lgT_sb = lgpool.tile([E, SG * P], F32, tag="lgT_sb")
                    nc.vector.tensor_copy(out=lgT_sb[:], in_=lgT[:])
                    nc.sync.dma_start(
                        out=lg_in[:, st * SG * P:(st + 1) * SG * P],
                        in_=lgT_sb[:])

            ag = nc.gpsimd.collective_compute(
                "AllGather", mybir.AluOpType.bypass,
                replica_groups=[list(range(n_cores))],
                ins=[lg_in[:]], outs=[lg_all[:]])

            # ---- weight pre-conversion fp32 -> bf16 (w1/w3 first, 2 queues) --
            conv_gated = set()
            for f in range(KF):
                for (wext, wdst, eng, tg) in ((w1_ext, w1bd, nc.sync, "a"),
                                              (w3_ext, w3bd, nc.scalar, "b")):
                    wst = spool.tile([P, KH, P], F32, tag="wst" + tg)
                    cld = eng.dma_start(
                        out=wst[:],
                        in_=wext[:, f * P:(f + 1) * P].rearrange(
                            "(k p) m -> p k m", p=P))
                    if tg not in conv_gated:
                        add_dep_helper(cld.ins, ag.ins, True, "conv after router")
                        conv_gated.add(tg)
                    wbt = spool.tile([P, KH * P], BF16, tag="wbt" + tg)
                    nc.vector.tensor_copy(
                        out=wbt[:].rearrange("p (k m) -> p k m", k=KH),
                        in_=wst[:])
                    eng.dma_start(out=wdst[f], in_=wbt[:])
            for f in range(KF):
                w2st = spool.tile([P, H], F32, tag="w2st")
                nc.sync.dma_start(out=w2st[:], in_=w2_ext[f * P:(f + 1) * P, :])
                nc.vector.tensor_copy(out=w2b[:, f], in_=w2st[:])

            # -------- combine + mask + bucket ranks for ALL tokens --------
            # long-lived outputs (kpool): rank_i, pays; the rest is scratch
            rank_i = kpool.tile([P, NT], I32)
            pays = kpool.tile([P, NT, 2], F32)
            with tc.tile_pool(name="cmb", bufs=1) as mpool, \
                 tc.tile_pool(name="cps", bufs=2, space="PSUM") as cpsum:
                lg_sb = mpool.tile([S * E, TS], F32)
                ldg = nc.scalar.dma_start(out=lg_sb[:], in_=lg_all[:])
                add_dep_helper(ldg.ins, ag.ins, True, "allgather done")
                # lgJ[p, j, s, e]: logit of expert e for token s*2048+j*128+p
                lgJ = mpool.tile([P, NTS, S, E], F32)
                for j in range(NTS):
                    lt = cpsum.tile([P, S * E], F32, tag="lt")
                    nc.tensor.transpose(out=lt[:],
                                        in_=lg_sb[:, j * P:(j + 1) * P],
                                        identity=ident[0:S * E, 0:S * E])
                    nc.scalar.copy(
                        out=lgJ[:, j],
                        in_=lt[:].rearrange("p (s e) -> p s e", s=S))

                # per-token top-8 of own-shard logits; nt = s*NTS + j
                mx = mpool.tile([P, NT, 8], F32)
                for j in range(NTS):
                    for s in range(S):
                        nc.vector.max(out=mx[:, s * NTS + j],
                                      in_=lgJ[:, j, s])

                # own-expert logit: le[:, s*NTS+j] = sum_e lgJ[:,j,s,e]*oh3[:,s,e]
                tmp4 = mpool.tile([P, NTS, S, E], F32)
                nc.vector.tensor_mul(
                    out=tmp4[:], in0=lgJ[:],
                    in1=oh3[:, None, :, :].to_broadcast([P, NTS, S, E]))
                leJ = mpool.tile([P, NTS, S], F32)
                nc.vector.tensor_reduce(out=leJ[:], in_=tmp4[:],
                                        axis=mybir.AxisListType.X,
                                        op=mybir.AluOpType.add)
                le = mpool.tile([P, NT], F32)
                for s in range(S):
                    nc.vector.tensor_copy(out=le[:, s * NTS:(s + 1) * NTS],
                                          in_=leJ[:, :, s])

                m1 = mx[:, :, 0]
                m2 = mx[:, :, 1]
                msum = mpool.tile([P, NT], F32)
                nc.vector.tensor_add(out=msum[:], in0=m1, in1=m2)
                sgin = mpool.tile([P, NT], F32)
                nc.vector.tensor_scalar_mul(sgin[:], le[:], 2.0)
                nc.vector.tensor_sub(out=sgin[:], in0=sgin[:], in1=msum[:])
                sig = mpool.tile([P, NT], F32)
                nc.scalar.activation(sig[:], sgin[:], AF.Sigmoid)
                eq1 = mpool.tile([P, NT], F32)
                eq2 = mpool.tile([P, NT], F32)
                nc.vector.tensor_tensor(out=eq1[:], in0=le[:], in1=m1,
                                        op=mybir.AluOpType.is_equal)
                nc.vector.tensor_tensor(out=eq2[:], in0=le[:], in1=m2,
                                        op=mybir.AluOpType.is_equal)
                mask = mpool.tile([P, NT], F32)
                nc.vector.tensor_add(out=mask[:], in0=eq1[:], in1=eq2[:])
                comb = mpool.tile([P, NT], F32)
                nc.vector.tensor_mul(out=comb[:], in0=mask[:], in1=sig[:])

                nc.vector.tensor_copy(out=pays[:, :, 0], in_=tok_slab[:])
                nc.vector.tensor_copy(out=pays[:, :, 1], in_=comb[:])

                # bucket-local ranks, all shards at once via segmented scan:
                # incl_j = zmask_j * incl_{j-1} + cs_j  (resets at shard starts)
                cs_row = mpool.tile([1, NT], F32)
                incl_row = mpool.tile([1, NT], F32)
                csum_ps = cpsum.tile([1, NT], F32, tag="c1")
                nc.tensor.matmul(csum_ps[:], lhsT=ones_col[:], rhs=mask[:],
                                 start=True, stop=True)
                nc.vector.tensor_copy(out=cs_row[:], in_=csum_ps[:])
                nc.vector.tensor_tensor_scan(out=incl_row[:],
                                             data0=zmask[:], data1=cs_row[:],
                                             initial=0.0,
                                             op0=mybir.AluOpType.mult,
                                             op1=mybir.AluOpType.add)
                cpref = mpool.tile([1, NT], F32)
                nc.vector.tensor_sub(out=cpref[:], in0=incl_row[:], in1=cs_row[:])
                nc.vector.tensor_add(out=cpref[:], in0=cpref[:], in1=bbase[:])
                rank_ps = cpsum.tile([P, NT], F32, tag="rk")
                nc.tensor.matmul(rank_ps[:], lhsT=ustrict[:], rhs=mask[:],
                                 start=True, stop=False)
                nc.tensor.matmul(rank_ps[:], lhsT=ones_row[:], rhs=cpref[:],
                                 start=False, stop=True)
                pad_off = mpool.tile([P, NT], F32)
                nc.vector.tensor_scalar(out=pad_off[:], in0=mask[:],
                                        scalar1=-BIG, scalar2=BIG,
                                        op0=mybir.AluOpType.mult,
                                        op1=mybir.AluOpType.add)
                rank_f = mpool.tile([P, NT], F32)
                nc.vector.tensor_add(out=rank_f[:], in0=rank_ps[:],
                                     in1=pad_off[:])
                nc.vector.tensor_copy(out=rank_i[:], in_=rank_f[:])

            # -------- FFN over compact slots, shard scatters interleaved ----
            fences = {}

            def emit_shard_scatters(s):
                shard_scatters = []
                for j in range(NTS):
                    nt = s * NTS + j
                    claim = bass.AP(
                        tensor=meta[0:P, :].tensor, offset=0,
                        ap=meta[0:P, :].ap,
                        dep_tracking_offset=(nt % NCT) * P * 2)
                    sc = nc.gpsimd.indirect_dma_start(
                        out=claim,
                        out_offset=bass.IndirectOffsetOnAxis(
                            ap=rank_i[:, nt:nt + 1], axis=0),
                        in_=pays[:, nt], in_offset=None,
                        bounds_check=TCAP - 1, oob_is_err=False)
                    sc.ins.queue = "qPoolDynamic" + str(nt % 4 or '')
                    shard_scatters.append(sc.ins)
                fence = nc.gpsimd.nop(hint=f"meta_fence_{s}", nofuse=True)
                for si in shard_scatters:
                    add_dep_helper(fence.ins, si, True, "meta scatter fence")
                fences[s] = fence

            rs_insts = {}
            out_scatter_insts = [[] for _ in range(S)]
            ZB = 4

            def emit_slab_zero(s):
                # zero scat rows [s*TS, (s+1)*TS) (+ trash rows with s == S-1)
                hi = (s + 1) * TS if s + 1 < S else T + P
                r = s * TS // P
                while r * P < hi:
                    n = min(ZB, hi // P - r)
                    nc.gpsimd.dma_start(
                        out=scat[r * P:(r + n) * P, :].rearrange(
                            "(a p) w -> p a w", p=P),
                        in_=zrow_b[:, None, :].to_broadcast([P, n, H]))
                    r += n
            with tc.tile_pool(name="hcp", bufs=CT + 2) as hcpool, \
                 tc.tile_pool(name="mtp", bufs=2) as mtpool, \
                 tc.tile_pool(name="fp", bufs=2) as fpool, \
                 tc.tile_pool(name="wp", bufs=4) as wpool, \
                 tc.tile_pool(name="gp", bufs=KF) as gpool, \
                 tc.tile_pool(name="op", bufs=3) as opool, \
                 tc.tile_pool(name="ftps", bufs=1, space="PSUM") as ftrpsum, \
                 tc.tile_pool(name="fps", bufs=2, space="PSUM") as fpsum, \
                 tc.tile_pool(name="ops", bufs=1, space="PSUM") as opsum:

                def emit_chunk_load(c):
                    """meta load + h gathers + PE transposes for chunk c."""
                    r0 = c * CH
                    lo_sh = r0 // BCAP
                    hi_sh = (r0 + CH - 1) // BCAP
                    mt = mtpool.tile([P, CT, 2], F32, tag="mt")
                    ld = nc.scalar.dma_start(
                        out=mt[:],
                        in_=meta[r0:r0 + CH, :].rearrange(
                            "(a p) w -> p a w", p=P))
                    for s in range(lo_sh, hi_sh + 1):
                        add_dep_helper(ld.ins, fences[s].ins, True, "meta fence")
                    idx = mtpool.tile([P, CT], I32, tag="idx")
                    nc.vector.tensor_copy(out=idx[:], in_=mt[:, :, 0])
                    idxg_f = mtpool.tile([P, CT], F32, tag="idxgf")
                    nc.vector.tensor_scalar_min(idxg_f[:], mt[:, :, 0],
                                                float(T - 1))
                    idxg = mtpool.tile([P, CT], I32, tag="idxg")
                    nc.vector.tensor_copy(out=idxg[:], in_=idxg_f[:])
                    hcts = []
                    for t in range(CT):
                        hct = hcpool.tile([P, H], F32, tag="hc")
                        g = nc.gpsimd.indirect_dma_start(
                            out=hct[:], out_offset=None,
                            in_=bass.AP(tensor=h_ext[0:P, :].tensor, offset=0,
                                        ap=h_ext[0:P, :].ap,
                                        dep_tracking_offset=0),
                            in_offset=bass.IndirectOffsetOnAxis(
                                ap=idxg[:, t:t + 1], axis=0),
                            bounds_check=T - 1, oob_is_err=False)
                        g.ins.queue = "qPoolDynamic" + str((c * CT + t) % 4 or '')
                        hcts.append(hct)
                    hTr = fpool.tile([P, KH, CH], BF16, tag="hTr")
                    for t in range(CT):
                        trp = ftrpsum.tile([P, KH, P], F32, tag="ftr")
                        for k in range(KH):
                            nc.tensor.transpose(
                                out=trp[:, k],
                                in_=hcts[t][:, k * P:(k + 1) * P],
                                identity=ident[:])
                        nc.scalar.copy(out=hTr[:, :, t * P:(t + 1) * P],
                                       in_=trp[:])
                    return mt, idx, hTr

                # prologue: shard 0 scatters, chunk 0 load, shard 1 scatters
                emit_slab_zero(0)
                emit_slab_zero(1)
                emit_shard_scatters(0)
                cur = emit_chunk_load(0)
                emit_shard_scatters(1)

                for c in range(NCH):
                    mt, idx, hTr = cur

                    # stage A: G^T tiles [f, tokens], bf16 streamed weights
                    gts = []
                    for f in range(KF):
                        w1s = wpool.tile([P, KH, P], BF16, tag="w1s")
                        nc.sync.dma_start(
                            out=w1s[:],
                            in_=w1bd[f].rearrange("p (k m) -> p k m", k=KH))
                        w3s = wpool.tile([P, KH, P], BF16, tag="w3s")
                        nc.sync.dma_start(
                            out=w3s[:],
                            in_=w3bd[f].rearrange("p (k m) -> p k m", k=KH))
                        x1 = fpsum.tile([P, CH], F32, tag="x1")
                        x3 = fpsum.tile([P, CH], F32, tag="x3")
                        for k in range(KH):
                            nc.tensor.matmul(x1[:], lhsT=w1s[:, k], rhs=hTr[:, k],
                                             start=(k == 0), stop=(k == KH - 1))
                        for k in range(KH):
                            nc.tensor.matmul(x3[:], lhsT=w3s[:, k], rhs=hTr[:, k],
                                             start=(k == 0), stop=(k == KH - 1))
                        gate = fpool.tile([P, CH], F32, tag="gate")
                        nc.scalar.activation(gate[:], x1[:], AF.Silu)
                        gt = gpool.tile([P, CH], BF16, tag="G")
                        nc.vector.tensor_mul(out=gt[:], in0=gate[:], in1=x3[:])
                        gts.append(gt)

                    # lookahead: next chunk's meta/gathers/transposes now, so
                    # its hTr copies complete during stage B; deferred shard
                    # scatters go after the gathers on the gpsimd engine
                    if c + 1 < NCH:
                        cur = emit_chunk_load(c + 1)
                    if c + 2 < S:
                        emit_slab_zero(c + 2)
                        emit_shard_scatters(c + 2)

                    # stage B: out rows, scaled by combine, scattered to scat
                    for t in range(CT):
                        o = opsum.tile([P, H], F32, tag="o")
                        for f in range(KF):
                            for hh in range(2):
                                nc.tensor.matmul(
                                    o[:, hh * 512:(hh + 1) * 512],
                                    lhsT=gts[f][:, t * P:(t + 1) * P],
                                    rhs=w2b[:, f, hh * 512:(hh + 1) * 512],
                                    start=(f == 0), stop=(f == KF - 1))
                        osb = opool.tile([P, H], BF16, tag="osb")
                        nc.vector.tensor_scalar_mul(osb[:], o[:], mt[:, t, 1:2])
                        gi = c * CT + t
                        lo_s = (gi * P) // BCAP
                        hi_s = min((gi * P + P - 1) // BCAP, S - 1)
                        if lo_s == hi_s:
                            doff = (lo_s * TS + (gi * P) % BCAP) * H
                        else:
                            doff = (hi_s * TS - P // 2) * H
                        oclaim = bass.AP(
                            tensor=scat[0:P, :].tensor, offset=0,
                            ap=scat[0:P, :].ap,
                            dep_tracking_offset=doff)
                        sco = nc.gpsimd.indirect_dma_start(
                            out=oclaim,
                            out_offset=bass.IndirectOffsetOnAxis(
                                ap=idx[:, t:t + 1], axis=0),
                            in_=osb[:], in_offset=None,
                            bounds_check=T + P - 1, oob_is_err=False)
                        sco.ins.queue = "qPoolDynamic" + str(gi % 4 or '')
                        for s in range(lo_s, hi_s + 1):
                            out_scatter_insts[s].append(sco.ins)

                    # issue slab ReduceScatters that became ready + out copies
                    for s in range(S):
                        if slab_ready_chunk[s] == c:
                            rs = nc.gpsimd.collective_compute(
                                "ReduceScatter", mybir.AluOpType.add,
                                replica_groups=[list(range(n_cores))],
                                ins=[scat[s * TS:(s + 1) * TS, :]],
                                outs=[rs_out[s * PS:(s + 1) * PS, :]])
                            for si in out_scatter_insts[s]:
                                add_dep_helper(rs.ins, si, True, "slab scatter")
                            rs_insts[s] = rs
                            for r in range(PS // P):
                                b0 = s * PS + r * P
                                oct_ = spool.tile([P, H], BF16, tag="oct")
                                ldo = nc.scalar.dma_start(
                                    out=oct_[:], in_=rs_out[b0:b0 + P, :])
                                add_dep_helper(ldo.ins, rs.ins, True, "rs done")
                                octf = spool.tile([P, H], F32, tag="octf")
                                nc.vector.tensor_copy(out=octf[:], in_=oct_[:])
                                nc.scalar.dma_start(out=out_ext[b0:b0 + P, :],
                                                    in_=octf[:])

    nc.finalize()
    return nc


def make_in_maps(hidden_states, gate_w, w1, w3, w2, n_cores=8):
    T, H = hidden_states.shape
    E = w1.shape[0]
    TS = T // n_cores
    h_full = np.ascontiguousarray(hidden_states, dtype=np.float32)
    in_maps = []
    for e in range(n_cores):
        oh64 = np.zeros(n_cores * E, dtype=np.float32)
        oh64[np.arange(n_cores) * E + e] = 1.0  # expert e, in every shard block
        in_maps.append({
            "h": h_full,
            "hs": np.ascontiguousarray(h_full[e * TS:(e + 1) * TS]),
            "gate_w": np.ascontiguousarray(gate_w, dtype=np.float32),
            "w1l": np.ascontiguousarray(w1[e], dtype=np.float32),
            "w3l": np.ascontiguousarray(w3[e], dtype=np.float32),
            "w2l": np.ascontiguousarray(w2[e], dtype=np.float32),
            "oh64": np.tile(oh64, (128, 1)),
        })
    return in_maps


def assemble_output(results, T=16384, H=1024, n_cores=8):
    # slab-RS s gives core r rows [s*2048 + r*256, s*2048 + (r+1)*256)
    S = n_cores
    TS = T // S
    PS = TS // S
    full = np.empty((T, H), dtype=np.float32)
    for r in range(n_cores):
        shard = np.asarray(results[r]["out_shard"])
        for s in range(S):
            full[s * TS + r * PS:s * TS + (r + 1) * PS] = \
                shard[s * PS:(s + 1) * PS]
    return full


def kernel(hidden_states, gate_w, w1, w3, w2):
    T, H = hidden_states.shape
    E, _, FF = w1.shape
    n_cores = 8
    nc = build_kernel(T=T, H=H, FF=FF, E=E, n_cores=n_cores)
    in_maps = make_in_maps(hidden_states, gate_w, w1, w3, w2, n_cores)
    res = run_bass_kernel_spmd(nc, in_maps, list(range(n_cores))).results
    return assemble_output(res, T=T, H=H, n_cores=n_cores)


if __name__ == "__main__":
    nc = build_kernel()
    print("built", len(nc.inst_map), "instructions")
